# revision 23
# baseline (speedup 1.0000x reference)
"""
Trainium2 Bass kernel for nn_Attention_29265907155069.

Reference computation (B=4, N=2048, C=768, H=12, D=64):
    qkv = x @ qkv_w.T -> split to q,k,v per head
    attn = softmax(q @ k.T * D + mask * -1e6)
    out  = (attn @ v) re-concat -> @ proj_w.T + proj_b

Sharding: 8 cores = (batch b in 0..3) x (head-group hg in 0..1, 6 heads each).
Each core computes its 6 heads' attention for its batch and a row-sharded
partial of the output projection; host sums the two head-group partials.

Per-core device pipeline:
  1. QKV: Q^T,K^T [d, n] and V [k, d] via PE matmuls (float32r). The D=64
     softmax scale is folded into Q on the host (Q-weights * 64).
  2. Scores S = 64*q@k.T per q-tile into PSUM (f32r), then an identity
     matmul accumulates -2^20 * mask (bf16, exact) onto the same PSUM tile.
  3. DVE reduce_max(negate) on PSUM -> -rowmax (masked entries sit at ~-1e6
     so the max is the masked row max).
  4. ACT: P = exp(S + (-rowmax)) from PSUM -> fp16 SBUF. Masked entries
     underflow to exactly 0, matching the reference's mask*-1e6 semantics.
  5. DMA xbar transpose P -> P^T (2-byte dtype, SBUF->SBUF).
  6. PV: O^T_unnorm[65, q] = [V | 1].T @ P^T accumulated over k tiles;
     row 64 = softmax denominators l (ones-column trick).
  7. recip(l) -> gpsimd partition_broadcast -> DVE mult => O^T normalized fp16.
  8. proj: Y[q, 768] = O^T.T @ projT (fp16) -> fp32 partial out.
"""

import os
import sys

import numpy as np

for _p in ("/opt/trn_rl_repo", "/root/.axon_site/_ro/trn_rl_repo"):
    if os.path.isdir(_p) and _p not in sys.path:
        sys.path.insert(0, _p)

import ml_dtypes  # noqa: E402

import concourse.mybir as mybir  # noqa: E402
from concourse import bacc  # noqa: E402
from concourse.bass_utils import run_bass_kernel_spmd  # noqa: E402
from concourse.masks import make_identity  # noqa: E402
from concourse.tile import TileContext  # noqa: E402

B, N, C, H = 4, 2048, 768, 12
D = C // H          # 64
HG = 2              # head groups (cores per batch)
HPC = H // HG       # heads per core = 6
CIN_T = C // 128    # 6 cin tiles
QT_TILES = 3        # 384 rows of Q^T (6 heads x 64) = 3 x 128
KT_TILES = N // 128  # 16
NCORES = 8
MASK_BIAS = -1048576.0  # -2^20, exact in bf16; scores are already x64

F32 = mybir.dt.float32
F32R = mybir.dt.float32r
F16 = mybir.dt.float16
BF16 = mybir.dt.bfloat16
F8E4 = mybir.dt.float8e4

# fp8 mask encoding (TRN2 f8e4 = IEEE e4m3, max +-240): ident diag = 128,
# mask entries = -128 -> product -16384 per masked score; scores are
# |s| <~ 3000 so masked entries sit at <= -13.5k, never the row max, and
# exp underflows to exactly 0.
IDENT_SCALE = 128.0
MASK_FP8_VAL = -128.0

_CACHE = {}


def _build_program(repeat=1):
    nc = bacc.Bacc(
        "TRN2",
        target_bir_lowering=False,
        debug=False,
        enable_asserts=False,
        num_devices=NCORES,
    )
    xT = nc.dram_tensor("xT", [C, N], F32R, kind="ExternalInput").ap()
    qkvT = nc.dram_tensor("qkvT", [C, 3 * HPC * D], F32R, kind="ExternalInput").ap()
    maskb = nc.dram_tensor("maskb", [N, N], F8E4, kind="ExternalInput").ap()
    projT = nc.dram_tensor("projT", [HPC * D, C], F16, kind="ExternalInput").ap()
    out = nc.dram_tensor("out", [N, C], F32, kind="ExternalOutput").ap()

    AL = mybir.AluOpType

    with TileContext(nc) as tc:
      for _rep in range(repeat):
        with tc.tile_pool(name="pers", bufs=1) as pers:
            # ---- persistent tiles ----
            QTs = [
                pers.tile([128, N], F32R, tag=f"qt{t}", name=f"qt{t}")
                for t in range(QT_TILES)
            ]
            KTs = [
                pers.tile([128, N], F32R, tag=f"kt{t}", name=f"kt{t}")
                for t in range(QT_TILES)
            ]
            # V augmented with a ones column: [128, (h,kt), 65]
            Vaug = pers.tile([128, HPC * KT_TILES, D + 1], F16, tag="vaug")
            Ocat = [
                pers.tile([128, N], F16, tag=f"oc{t}", name=f"oc{t}")
                for t in range(QT_TILES)
            ]
            PW = [
                pers.tile([128, C], F16, tag=f"pw{t}", name=f"pw{t}")
                for t in range(QT_TILES)
            ]
            identb = pers.tile([128, 128], BF16, tag="identb")
            ident = pers.tile([128, 128], F8E4, tag="ident")
            ones64 = pers.tile([1, D], F32, tag="ones64")

            make_identity(nc, identb[:, :])
            nc.scalar.mul(ident[:, :], identb[:, :], IDENT_SCALE)
            nc.vector.memset(ones64[:, :], 1.0)
            nc.gpsimd.memset(Vaug[:, :, D : D + 1], 1.0)
            for t in range(QT_TILES):
                nc.sync.dma_start(PW[t][:, :], projT[t * 128 : (t + 1) * 128, :])

            # ================= Phase 1: QKV projection =================
            with (
                tc.tile_pool(name="ph1", bufs=1) as p1,
                tc.tile_pool(name="ph1p", bufs=4, space="PSUM") as p1p,
            ):
                xts = [
                    p1.tile([128, N], F32R, tag=f"x{ci}", name=f"x{ci}")
                    for ci in range(CIN_T)
                ]
                wts = [
                    p1.tile([128, 3 * HPC * D], F32R, tag=f"w{ci}", name=f"w{ci}")
                    for ci in range(CIN_T)
                ]
                # load order: K weight cols + first x chunk first so the K
                # matmuls can start ~8us in instead of ~13us.
                koff = HPC * D
                for ci in range(CIN_T):
                    nc.scalar.dma_start(
                        wts[ci][:, koff : 2 * koff],
                        qkvT[ci * 128 : (ci + 1) * 128, koff : 2 * koff],
                    )
                    nc.sync.dma_start(
                        xts[ci][:, 0:512], xT[ci * 128 : (ci + 1) * 128, 0:512]
                    )
                for ci in range(CIN_T):
                    nc.scalar.dma_start(
                        wts[ci][:, 0:koff], qkvT[ci * 128 : (ci + 1) * 128, 0:koff]
                    )
                for qc in range(1, 4):
                    for ci in range(CIN_T):
                        nc.sync.dma_start(
                            xts[ci][:, qc * 512 : (qc + 1) * 512],
                            xT[ci * 128 : (ci + 1) * 128, qc * 512 : (qc + 1) * 512],
                        )
                for ci in range(CIN_T):
                    nc.scalar.dma_start(
                        wts[ci][:, 2 * koff : 3 * koff],
                        qkvT[ci * 128 : (ci + 1) * 128, 2 * koff : 3 * koff],
                    )

                # K^T then Q^T production: out[d_tile 128, q 512]
                for which, dst in ((1, KTs), (0, QTs)):
                    off = which * HPC * D  # 0 or 384 within qkvT cols
                    for qc in range(4):
                        for t in range(QT_TILES):
                            ps = p1p.tile([128, 512], F32, tag="p1ps", name="ps")
                            for ci in range(CIN_T):
                                nc.tensor.matmul(
                                    ps[:, :],
                                    wts[ci][:, off + t * 128 : off + (t + 1) * 128],
                                    xts[ci][:, qc * 512 : (qc + 1) * 512],
                                    start=(ci == 0),
                                    stop=(ci == CIN_T - 1),
                                )
                            nc.scalar.copy(
                                dst[t][:, qc * 512 : (qc + 1) * 512], ps[:, :]
                            )

                # V production: out[k_tile 128, 384] -> Vaug f16 (strided per head)
                voff = 2 * HPC * D  # 768
                for kt in range(KT_TILES):
                    ps = p1p.tile([128, HPC * D], F32, tag="p1ps", name="ps")
                    for ci in range(CIN_T):
                        nc.tensor.matmul(
                            ps[:, :],
                            xts[ci][:, kt * 128 : (kt + 1) * 128],
                            wts[ci][:, voff : voff + HPC * D],
                            start=(ci == 0),
                            stop=(ci == CIN_T - 1),
                        )
                    # psum [128, (h 6, d 64)] -> Vaug[:, h*16+kt, 0:64]
                    nc.scalar.copy(
                        Vaug[:, kt :: KT_TILES, 0:D],
                        ps[:, :].rearrange("p (h d) -> p h d", h=HPC),
                    )

            # ================= Phase 2: attention =================
            # Software-pipelined emission: PV groups lag the score/softmax
            # stream by PV_LAG score-groups so the DVE always has reduce work
            # queued while the PE chews through PV/proj matmuls.
            PV_LAG = 3
            with (
                tc.tile_pool(name="mk", bufs=2) as pmk,
                tc.tile_pool(name="work", bufs=2) as pw,
                tc.tile_pool(name="psS", bufs=3, space="PSUM") as psS,
                tc.tile_pool(name="psO", bufs=2, space="PSUM") as psO,
            ):
                mks = {}     # qc -> list of 4 mask tiles
                PT = {}      # (qc, hp) -> [PT0, PT1]

                def load_masks(qc):
                    # scalar-engine HWDGE queue: keeps mask loads from
                    # head-of-line blocking the transposes on the sync queue
                    tiles = []
                    for j in range(4):
                        mk = pmk.tile([128, N], F8E4, tag=f"mk{j}", name=f"mk{j}")
                        row0 = qc * 512 + j * 128
                        nc.sync.dma_start(mk[:, :], maskb[row0 : row0 + 128, :])
                        tiles.append(mk)
                    mks[qc] = tiles

                def score_group(qc, hp, j):
                    if j == 0:
                        PT[(qc, hp)] = [
                            pw.tile(
                                [128, KT_TILES, 512], F16, tag="ptrans",
                                name=f"PT{a}", bufs=4,
                            )
                            for a in range(2)
                        ]
                    PTs = PT[(qc, hp)]
                    qt = qc * 4 + j
                    mstats = pw.tile([128, 4], F32, tag="mstat", name="mstat",
                                     bufs=6)
                    negm = pw.tile([128, 2], F32, tag="negm", name="negm",
                                   bufs=6)
                    fs = pw.tile([128, 2], F32, tag="fs", name="fs", bufs=6)
                    pns = [
                        pw.tile([128, N], F16, tag=f"pn{a}", name=f"pn{a}",
                                bufs=3)
                        for a in range(2)
                    ]
                    for half in range(2):
                        sps = [
                            psS.tile([128, 1024], F32, tag="sp", name=f"sp{a}")
                            for a in range(2)
                        ]
                        # packed K=64 score matmuls: head a in row group a
                        for c in range(2):
                            kc = half * 1024 + c * 512
                            for a in range(2):
                                nc.tensor.matmul(
                                    sps[a][:, c * 512 : (c + 1) * 512],
                                    QTs[hp][
                                        a * D : (a + 1) * D,
                                        qt * 128 : (qt + 1) * 128,
                                    ],
                                    KTs[hp][a * D : (a + 1) * D, kc : kc + 512],
                                    start=True,
                                    stop=False,
                                    tile_position=(a * D, 0),
                                )
                        for c in range(2):
                            kc = half * 1024 + c * 512
                            for a in range(2):
                                nc.tensor.matmul(
                                    sps[a][:, c * 512 : (c + 1) * 512],
                                    ident[:, :],
                                    mks[qc][j][:, kc : kc + 512],
                                    start=False,
                                    stop=True,
                                )
                        for a in range(2):
                            # mstats layout: [a0h0, a0h1, a1h0, a1h1]
                            col = a * 2 + half
                            nc.vector.tensor_reduce(
                                mstats[:, col : col + 1],
                                sps[a][:, :],
                                axis=mybir.AxisListType.X,
                                op=AL.max,
                                negate=True,
                            )
                            if half == 0:
                                # early exp with the half-0 max; fs0
                                # correction lands after negm is known
                                nc.scalar.activation(
                                    pns[a][:, 0:1024],
                                    sps[a][:, :],
                                    mybir.ActivationFunctionType.Exp,
                                    bias=mstats[:, 2 * a : 2 * a + 1],
                                    scale=1.0,
                                )
                            else:
                                # negm[a] = min over (h0, h1) for head a:
                                # strided segmented reduce [128,(2h,2a)] view
                                nc.vector.tensor_reduce(
                                    negm[:, a : a + 1],
                                    mstats[:, 2 * a : 2 * a + 2],
                                    axis=mybir.AxisListType.X,
                                    op=AL.min,
                                )
                                # fs0 = exp(m0 - m) first so the half-0
                                # rescale isn't queued behind the big exp
                                nc.scalar.activation(
                                    fs[:, a : a + 1],
                                    mstats[:, 2 * a : 2 * a + 1],
                                    mybir.ActivationFunctionType.Exp,
                                    bias=negm[:, a : a + 1],
                                    scale=-1.0,
                                )
                                nc.scalar.activation(
                                    pns[a][:, 1024:2048],
                                    sps[a][:, :],
                                    mybir.ActivationFunctionType.Exp,
                                    bias=negm[:, a : a + 1],
                                    scale=1.0,
                                )
                                nc.vector.tensor_scalar(
                                    pns[a][:, 0:1024],
                                    pns[a][:, 0:1024],
                                    fs[:, a : a + 1],
                                    None,
                                    op0=AL.mult,
                                )
                                nc.sync.dma_start_transpose(
                                    PTs[a][:, :, j * 128 : (j + 1) * 128],
                                    pns[a][:, :],
                                )

                def pv_group(qc, hp, a):
                    PTs = PT[(qc, hp)]
                    h = 2 * hp + a
                    ht, hpp = hp, a * D
                    # PV: O^T_unnorm [65, 512q]
                    ot = psO.tile([D + 1, 512], F32, tag="ot", name="ot")
                    for kt in range(KT_TILES):
                        nc.tensor.matmul(
                            ot[:, :],
                            Vaug[:, h * KT_TILES + kt, :],
                            PTs[a][:, kt, :],
                            start=(kt == 0),
                            stop=(kt == KT_TILES - 1),
                        )
                    rl = pw.tile([1, 512], F32, tag="rl", name="rl", bufs=4)
                    nc.vector.reciprocal(rl[:, :], ot[D : D + 1, :])
                    rb = pw.tile([D, 512], F32, tag="rb", name="rb", bufs=4)
                    nc.gpsimd.partition_broadcast(rb[:, :], rl[:, :])
                    nc.vector.tensor_tensor(
                        Ocat[ht][hpp : hpp + D, qc * 512 : (qc + 1) * 512],
                        ot[0:D, :],
                        rb[:, :],
                        op=AL.mult,
                    )
                    if a == 1:
                        PT.pop((qc, hp))

                def proj_group(qc, j):
                    qt = qc * 4 + j
                    y0 = psO.tile([128, 512], F32, tag="ot", name="y0")
                    y1 = psO.tile([128, 256], F32, tag="ot", name="y1")
                    for ct in range(QT_TILES):
                        lt = Ocat[ct][:, qt * 128 : (qt + 1) * 128]
                        nc.tensor.matmul(
                            y0[:, :], lt, PW[ct][:, 0:512],
                            start=(ct == 0), stop=(ct == QT_TILES - 1),
                        )
                        nc.tensor.matmul(
                            y1[:, :], lt, PW[ct][:, 512:768],
                            start=(ct == 0), stop=(ct == QT_TILES - 1),
                        )
                    ysb = pw.tile([128, C], F32, tag="ysb", name="ysb")
                    nc.scalar.copy(ysb[:, 0:512], y0[:, :])
                    nc.scalar.copy(ysb[:, 512:768], y1[:, :])
                    nc.sync.dma_start(out[qt * 128 : (qt + 1) * 128, :], ysb[:, :])

                # flat schedule: score groups in (qc, hp, j) order; each
                # deferred task fires PV_LAG group-slots after its data is
                # complete.
                groups = [
                    (qc, hp, j)
                    for qc in range(4)
                    for hp in range(QT_TILES)
                    for j in range(4)
                ]
                pending = []  # (due_slot, fn)
                load_masks(0)
                for slot, (qc, hp, j) in enumerate(groups):
                    if j == 0 and hp == 1 and qc + 1 < 4:
                        load_masks(qc + 1)  # prefetch next q-chunk's masks
                    for due, fn in [p for p in pending if p[0] <= slot]:
                        fn()
                        pending.remove((due, fn))
                    score_group(qc, hp, j)
                    if j == 3:
                        # the last head-pair's PV can lag further: its PT ring
                        # slot isn't needed until deep into the next q-chunk
                        lag = PV_LAG
                        for a in range(2):
                            pending.append(
                                (
                                    slot + lag + a,
                                    lambda qc=qc, hp=hp, a=a: pv_group(qc, hp, a),
                                )
                            )
                        if hp == QT_TILES - 1:
                            for jj in range(4):
                                pending.append(
                                    (
                                        slot + lag + 2 + jj,
                                        lambda qc=qc, jj=jj: proj_group(qc, jj),
                                    )
                                )
                for due, fn in sorted(pending, key=lambda p: p[0]):
                    fn()
    nc.compile()
    return nc


def _prepare_in_maps(x, local_attn_mask, qkv_w, proj_w, proj_b):
    x = np.asarray(x, dtype=np.float32)
    mask = np.asarray(local_attn_mask)
    qkv_w = np.asarray(qkv_w, dtype=np.float32)
    proj_w = np.asarray(proj_w, dtype=np.float32)

    maskb = (MASK_FP8_VAL * mask.astype(np.float32)).astype(ml_dtypes.float8_e4m3)
    in_maps = []
    for c in range(NCORES):
        b, hg = c // HG, c % HG
        rq = slice(hg * HPC * D, (hg + 1) * HPC * D)
        rk = slice(C + hg * HPC * D, C + (hg + 1) * HPC * D)
        rv = slice(2 * C + hg * HPC * D, 2 * C + (hg + 1) * HPC * D)
        # softmax scale D folded into the Q weights
        wsel = np.concatenate(
            [qkv_w[rq] * float(D), qkv_w[rk], qkv_w[rv]], axis=0
        )  # [1152, 768]
        in_maps.append(
            {
                "xT": np.ascontiguousarray(x[b].T),
                "qkvT": np.ascontiguousarray(wsel.T),
                "maskb": maskb,
                "projT": np.ascontiguousarray(
                    proj_w[:, hg * HPC * D : (hg + 1) * HPC * D].T
                ).astype(np.float16),
            }
        )
    return in_maps


def kernel(x, local_attn_mask, qkv_w, proj_w, proj_b):
    proj_b = np.asarray(proj_b, dtype=np.float32)
    in_maps = _prepare_in_maps(x, local_attn_mask, qkv_w, proj_w, proj_b)

    if "nc" not in _CACHE:
        _CACHE["nc"] = _build_program()
    res = run_bass_kernel_spmd(
        _CACHE["nc"],
        in_maps,
        core_ids=list(range(NCORES)),
        tmpdir=os.environ.get("KPROF_DIR") or None,
    )
    _CACHE["last_result"] = res
    outs = res.results
    y = np.empty((B, N, C), dtype=np.float32)
    for b in range(B):
        y[b] = outs[2 * b]["out"] + outs[2 * b + 1]["out"] + proj_b[None, :]
    return y



# revision 26
# speedup vs baseline: 1.3573x; 1.3573x over previous
"""
Trainium2 Bass kernel for nn_Attention_29265907155069.

Reference computation (B=4, N=2048, C=768, H=12, D=64):
    qkv = x @ qkv_w.T -> split to q,k,v per head
    attn = softmax(q @ k.T * D + mask * -1e6)
    out  = (attn @ v) re-concat -> @ proj_w.T + proj_b

Sharding: 8 cores = (batch b in 0..3) x (head-group hg in 0..1, 6 heads each).
Each core computes its 6 heads' attention for its batch and a row-sharded
partial of the output projection; host sums the two head-group partials.

Per-core device pipeline:
  1. QKV: Q^T,K^T [d, n] and V [k, d] via PE matmuls (float32r). The D=64
     softmax scale is folded into Q on the host (Q-weights * 64).
  2. Scores S = 64*q@k.T per q-tile into PSUM (f32r), then an identity
     matmul accumulates -2^20 * mask (bf16, exact) onto the same PSUM tile.
  3. DVE reduce_max(negate) on PSUM -> -rowmax (masked entries sit at ~-1e6
     so the max is the masked row max).
  4. ACT: P = exp(S + (-rowmax)) from PSUM -> fp16 SBUF. Masked entries
     underflow to exactly 0, matching the reference's mask*-1e6 semantics.
  5. DMA xbar transpose P -> P^T (2-byte dtype, SBUF->SBUF).
  6. PV: O^T_unnorm[65, q] = [V | 1].T @ P^T accumulated over k tiles;
     row 64 = softmax denominators l (ones-column trick).
  7. recip(l) -> gpsimd partition_broadcast -> DVE mult => O^T normalized fp16.
  8. proj: Y[q, 768] = O^T.T @ projT (fp16) -> fp32 partial out.
"""

import os
import sys

import numpy as np

for _p in ("/opt/trn_rl_repo", "/root/.axon_site/_ro/trn_rl_repo"):
    if os.path.isdir(_p) and _p not in sys.path:
        sys.path.insert(0, _p)

import ml_dtypes  # noqa: E402

import concourse.mybir as mybir  # noqa: E402
from concourse import bacc  # noqa: E402
from concourse.bass_utils import run_bass_kernel_spmd  # noqa: E402
from concourse.masks import make_identity  # noqa: E402
from concourse.tile import TileContext  # noqa: E402

B, N, C, H = 4, 2048, 768, 12
D = C // H          # 64
HG = 2              # head groups (cores per batch)
HPC = H // HG       # heads per core = 6
CIN_T = C // 128    # 6 cin tiles
QT_TILES = 3        # 384 rows of Q^T (6 heads x 64) = 3 x 128
KT_TILES = N // 128  # 16
NCORES = 8
MASK_BIAS = -1048576.0  # -2^20, exact in bf16; scores are already x64

F32 = mybir.dt.float32
F32R = mybir.dt.float32r
F16 = mybir.dt.float16
BF16 = mybir.dt.bfloat16
F8E4 = mybir.dt.float8e4

# fp8 mask encoding (TRN2 f8e4 = IEEE e4m3, max +-240): ident diag = 128,
# mask entries = -128 -> product -16384 per masked score; scores are
# |s| <~ 3000 so masked entries sit at <= -13.5k, never the row max, and
# exp underflows to exactly 0.
IDENT_SCALE = 128.0
MASK_FP8_VAL = -128.0

_CACHE = {}


def _build_program(repeat=1):
    nc = bacc.Bacc(
        "TRN2",
        target_bir_lowering=False,
        debug=False,
        enable_asserts=False,
        num_devices=NCORES,
    )
    xT = nc.dram_tensor("xT", [C, N], F32R, kind="ExternalInput").ap()
    qkvT = nc.dram_tensor("qkvT", [C, 3 * HPC * D], F32R, kind="ExternalInput").ap()
    maskb = nc.dram_tensor("maskb", [N, N], F8E4, kind="ExternalInput").ap()
    projT = nc.dram_tensor("projT", [HPC * D, C], F16, kind="ExternalInput").ap()
    out = nc.dram_tensor("out", [N, C], F32, kind="ExternalOutput").ap()

    AL = mybir.AluOpType

    with TileContext(nc) as tc:
      for _rep in range(repeat):
        with tc.tile_pool(name="pers", bufs=1) as pers:
            # ---- persistent tiles ----
            QTs = [
                pers.tile([128, N], F32R, tag=f"qt{t}", name=f"qt{t}")
                for t in range(QT_TILES)
            ]
            KTs = [
                pers.tile([128, N], F32R, tag=f"kt{t}", name=f"kt{t}")
                for t in range(QT_TILES)
            ]
            # V augmented with a ones column: [128, (h,kt), 65]
            Vaug = pers.tile([128, HPC * KT_TILES, D + 1], F16, tag="vaug")
            Ocat = [
                pers.tile([128, N], F16, tag=f"oc{t}", name=f"oc{t}")
                for t in range(QT_TILES)
            ]
            PW = [
                pers.tile([128, C], F16, tag=f"pw{t}", name=f"pw{t}")
                for t in range(QT_TILES)
            ]
            identb = pers.tile([128, 128], BF16, tag="identb")
            ident = pers.tile([128, 128], F8E4, tag="ident")
            ones64 = pers.tile([1, D], F32, tag="ones64")

            make_identity(nc, identb[:, :])
            nc.scalar.mul(ident[:, :], identb[:, :], IDENT_SCALE)
            nc.vector.memset(ones64[:, :], 1.0)
            nc.gpsimd.memset(Vaug[:, :, D : D + 1], 1.0)
            for t in range(QT_TILES):
                nc.sync.dma_start(PW[t][:, :], projT[t * 128 : (t + 1) * 128, :])

            # ================= Phase 1: QKV projection =================
            with (
                tc.tile_pool(name="ph1", bufs=1) as p1,
                tc.tile_pool(name="ph1p", bufs=4, space="PSUM") as p1p,
            ):
                xts = [
                    p1.tile([128, N], F32R, tag=f"x{ci}", name=f"x{ci}")
                    for ci in range(CIN_T)
                ]
                wts = [
                    p1.tile([128, 3 * HPC * D], F32R, tag=f"w{ci}", name=f"w{ci}")
                    for ci in range(CIN_T)
                ]
                # load order: K weight cols + first x chunk first so the K
                # matmuls can start ~8us in instead of ~13us.
                koff = HPC * D
                for ci in range(CIN_T):
                    nc.scalar.dma_start(
                        wts[ci][:, koff : 2 * koff],
                        qkvT[ci * 128 : (ci + 1) * 128, koff : 2 * koff],
                    )
                    nc.sync.dma_start(
                        xts[ci][:, 0:512], xT[ci * 128 : (ci + 1) * 128, 0:512]
                    )
                for ci in range(CIN_T):
                    nc.scalar.dma_start(
                        wts[ci][:, 0:koff], qkvT[ci * 128 : (ci + 1) * 128, 0:koff]
                    )
                for qc in range(1, 4):
                    for ci in range(CIN_T):
                        nc.sync.dma_start(
                            xts[ci][:, qc * 512 : (qc + 1) * 512],
                            xT[ci * 128 : (ci + 1) * 128, qc * 512 : (qc + 1) * 512],
                        )
                for ci in range(CIN_T):
                    nc.scalar.dma_start(
                        wts[ci][:, 2 * koff : 3 * koff],
                        qkvT[ci * 128 : (ci + 1) * 128, 2 * koff : 3 * koff],
                    )

                # K^T then Q^T production: out[d_tile 128, q 512]
                for which, dst in ((1, KTs), (0, QTs)):
                    off = which * HPC * D  # 0 or 384 within qkvT cols
                    for qc in range(4):
                        for t in range(QT_TILES):
                            ps = p1p.tile([128, 512], F32, tag="p1ps", name="ps")
                            for ci in range(CIN_T):
                                nc.tensor.matmul(
                                    ps[:, :],
                                    wts[ci][:, off + t * 128 : off + (t + 1) * 128],
                                    xts[ci][:, qc * 512 : (qc + 1) * 512],
                                    start=(ci == 0),
                                    stop=(ci == CIN_T - 1),
                                )
                            nc.scalar.copy(
                                dst[t][:, qc * 512 : (qc + 1) * 512], ps[:, :]
                            )

                # V production: out[k_tile 128, 384] -> Vaug f16 (strided per head)
                voff = 2 * HPC * D  # 768
                for kt in range(KT_TILES):
                    ps = p1p.tile([128, HPC * D], F32, tag="p1ps", name="ps")
                    for ci in range(CIN_T):
                        nc.tensor.matmul(
                            ps[:, :],
                            xts[ci][:, kt * 128 : (kt + 1) * 128],
                            wts[ci][:, voff : voff + HPC * D],
                            start=(ci == 0),
                            stop=(ci == CIN_T - 1),
                        )
                    # psum [128, (h 6, d 64)] -> Vaug[:, h*16+kt, 0:64]
                    nc.scalar.copy(
                        Vaug[:, kt :: KT_TILES, 0:D],
                        ps[:, :].rearrange("p (h d) -> p h d", h=HPC),
                    )

            # ================= Phase 2: attention =================
            # Software-pipelined emission: PV groups lag the score/softmax
            # stream by PV_LAG score-groups so the DVE always has reduce work
            # queued while the PE chews through PV/proj matmuls.
            PV_LAG = 3
            with (
                tc.tile_pool(name="mk", bufs=2) as pmk,
                tc.tile_pool(name="work", bufs=2) as pw,
                tc.tile_pool(name="psS", bufs=3, space="PSUM") as psS,
                tc.tile_pool(name="psO", bufs=2, space="PSUM") as psO,
            ):
                mks = {}     # qc -> list of 4 mask tiles
                PT = {}      # (qc, hp) -> [PT0, PT1]

                def load_masks(qc):
                    # scalar-engine HWDGE queue: keeps mask loads from
                    # head-of-line blocking the transposes on the sync queue
                    tiles = []
                    for j in range(4):
                        mk = pmk.tile([128, N], F8E4, tag=f"mk{j}", name=f"mk{j}")
                        row0 = qc * 512 + j * 128
                        nc.sync.dma_start(mk[:, :], maskb[row0 : row0 + 128, :])
                        tiles.append(mk)
                    mks[qc] = tiles

                def score_group(qc, hp, j):
                    if j == 0:
                        PT[(qc, hp)] = [
                            pw.tile(
                                [128, KT_TILES, 512], F16, tag="ptrans",
                                name=f"PT{a}", bufs=4,
                            )
                            for a in range(2)
                        ]
                    PTs = PT[(qc, hp)]
                    qt = qc * 4 + j
                    mstats = pw.tile([128, 4], F32, tag="mstat", name="mstat",
                                     bufs=6)
                    negm = pw.tile([128, 2], F32, tag="negm", name="negm",
                                   bufs=6)
                    fs = pw.tile([128, 2], F32, tag="fs", name="fs", bufs=6)
                    pns = [
                        pw.tile([128, N], F16, tag=f"pn{a}", name=f"pn{a}",
                                bufs=3)
                        for a in range(2)
                    ]
                    for half in range(2):
                        sps = [
                            psS.tile([128, 1024], F32, tag="sp", name=f"sp{a}")
                            for a in range(2)
                        ]
                        # packed K=64 score matmuls: head a in row group a
                        for c in range(2):
                            kc = half * 1024 + c * 512
                            for a in range(2):
                                nc.tensor.matmul(
                                    sps[a][:, c * 512 : (c + 1) * 512],
                                    QTs[hp][
                                        a * D : (a + 1) * D,
                                        qt * 128 : (qt + 1) * 128,
                                    ],
                                    KTs[hp][a * D : (a + 1) * D, kc : kc + 512],
                                    start=True,
                                    stop=False,
                                    tile_position=(a * D, 0),
                                )
                        for c in range(2):
                            kc = half * 1024 + c * 512
                            for a in range(2):
                                nc.tensor.matmul(
                                    sps[a][:, c * 512 : (c + 1) * 512],
                                    ident[:, :],
                                    mks[qc][j][:, kc : kc + 512],
                                    start=False,
                                    stop=True,
                                )
                        for a in range(2):
                            # mstats layout: [a0h0, a0h1, a1h0, a1h1]
                            col = a * 2 + half
                            nc.vector.tensor_reduce(
                                mstats[:, col : col + 1],
                                sps[a][:, :],
                                axis=mybir.AxisListType.X,
                                op=AL.max,
                                negate=True,
                            )
                            if half == 0:
                                # early exp with the half-0 max; fs0
                                # correction lands after negm is known
                                nc.scalar.activation(
                                    pns[a][:, 0:1024],
                                    sps[a][:, :],
                                    mybir.ActivationFunctionType.Exp,
                                    bias=mstats[:, 2 * a : 2 * a + 1],
                                    scale=1.0,
                                )
                        if half == 1:
                            for a in range(2):
                                nc.vector.tensor_reduce(
                                    negm[:, a : a + 1],
                                    mstats[:, 2 * a : 2 * a + 2],
                                    axis=mybir.AxisListType.X,
                                    op=AL.min,
                                )
                                # fs0 = exp(m0 - m) first so the half-0
                                # rescale isn't queued behind the big exp
                                nc.scalar.activation(
                                    fs[:, a : a + 1],
                                    mstats[:, 2 * a : 2 * a + 1],
                                    mybir.ActivationFunctionType.Exp,
                                    bias=negm[:, a : a + 1],
                                    scale=-1.0,
                                )
                                nc.scalar.activation(
                                    pns[a][:, 1024:2048],
                                    sps[a][:, :],
                                    mybir.ActivationFunctionType.Exp,
                                    bias=negm[:, a : a + 1],
                                    scale=1.0,
                                )
                                nc.vector.tensor_scalar(
                                    pns[a][:, 0:1024],
                                    pns[a][:, 0:1024],
                                    fs[:, a : a + 1],
                                    None,
                                    op0=AL.mult,
                                )
                                nc.sync.dma_start_transpose(
                                    PTs[a][:, 0:8, j * 128 : (j + 1) * 128],
                                    pns[a][:, 0:1024],
                                )
                                nc.sync.dma_start_transpose(
                                    PTs[a][:, 8:16, j * 128 : (j + 1) * 128],
                                    pns[a][:, 1024:2048],
                                )

                def pv_group(qc, hp, a):
                    PTs = PT[(qc, hp)]
                    h = 2 * hp + a
                    ht, hpp = hp, a * D
                    # PV: O^T_unnorm [65, 512q]
                    ot = psO.tile([D + 1, 512], F32, tag="ot", name="ot")
                    for kt in range(KT_TILES):
                        nc.tensor.matmul(
                            ot[:, :],
                            Vaug[:, h * KT_TILES + kt, :],
                            PTs[a][:, kt, :],
                            start=(kt == 0),
                            stop=(kt == KT_TILES - 1),
                        )
                    rl = pw.tile([1, 512], F32, tag="rl", name="rl", bufs=4)
                    nc.vector.reciprocal(rl[:, :], ot[D : D + 1, :])
                    rb = pw.tile([D, 512], F32, tag="rb", name="rb", bufs=4)
                    nc.gpsimd.partition_broadcast(rb[:, :], rl[:, :])
                    nc.vector.tensor_tensor(
                        Ocat[ht][hpp : hpp + D, qc * 512 : (qc + 1) * 512],
                        ot[0:D, :],
                        rb[:, :],
                        op=AL.mult,
                    )
                    if a == 1:
                        PT.pop((qc, hp))

                def proj_group(qc, j):
                    qt = qc * 4 + j
                    y0 = psO.tile([128, 512], F32, tag="ot", name="y0")
                    y1 = psO.tile([128, 256], F32, tag="ot", name="y1")
                    for ct in range(QT_TILES):
                        lt = Ocat[ct][:, qt * 128 : (qt + 1) * 128]
                        nc.tensor.matmul(
                            y0[:, :], lt, PW[ct][:, 0:512],
                            start=(ct == 0), stop=(ct == QT_TILES - 1),
                        )
                        nc.tensor.matmul(
                            y1[:, :], lt, PW[ct][:, 512:768],
                            start=(ct == 0), stop=(ct == QT_TILES - 1),
                        )
                    ysb = pw.tile([128, C], F32, tag="ysb", name="ysb")
                    nc.scalar.copy(ysb[:, 0:512], y0[:, :])
                    nc.scalar.copy(ysb[:, 512:768], y1[:, :])
                    nc.sync.dma_start(out[qt * 128 : (qt + 1) * 128, :], ysb[:, :])

                # flat schedule: score groups in (qc, hp, j) order; each
                # deferred task fires PV_LAG group-slots after its data is
                # complete.
                groups = [
                    (qc, hp, j)
                    for qc in range(4)
                    for hp in range(QT_TILES)
                    for j in range(4)
                ]
                pending = []  # (due_slot, fn)
                load_masks(0)
                for slot, (qc, hp, j) in enumerate(groups):
                    if j == 0 and hp == 1 and qc + 1 < 4:
                        load_masks(qc + 1)  # prefetch next q-chunk's masks
                    for due, fn in [p for p in pending if p[0] <= slot]:
                        fn()
                        pending.remove((due, fn))
                    score_group(qc, hp, j)
                    if j == 3:
                        # the last head-pair's PV can lag further: its PT ring
                        # slot isn't needed until deep into the next q-chunk
                        lag = PV_LAG
                        for a in range(2):
                            pending.append(
                                (
                                    slot + lag + a,
                                    lambda qc=qc, hp=hp, a=a: pv_group(qc, hp, a),
                                )
                            )
                        if hp == QT_TILES - 1:
                            for jj in range(4):
                                pending.append(
                                    (
                                        slot + lag + 2 + jj,
                                        lambda qc=qc, jj=jj: proj_group(qc, jj),
                                    )
                                )
                for due, fn in sorted(pending, key=lambda p: p[0]):
                    fn()
    nc.compile()
    return nc


def _prepare_in_maps(x, local_attn_mask, qkv_w, proj_w, proj_b):
    x = np.asarray(x, dtype=np.float32)
    mask = np.asarray(local_attn_mask)
    qkv_w = np.asarray(qkv_w, dtype=np.float32)
    proj_w = np.asarray(proj_w, dtype=np.float32)

    maskb = (MASK_FP8_VAL * mask.astype(np.float32)).astype(ml_dtypes.float8_e4m3)
    in_maps = []
    for c in range(NCORES):
        b, hg = c // HG, c % HG
        rq = slice(hg * HPC * D, (hg + 1) * HPC * D)
        rk = slice(C + hg * HPC * D, C + (hg + 1) * HPC * D)
        rv = slice(2 * C + hg * HPC * D, 2 * C + (hg + 1) * HPC * D)
        # softmax scale D folded into the Q weights
        wsel = np.concatenate(
            [qkv_w[rq] * float(D), qkv_w[rk], qkv_w[rv]], axis=0
        )  # [1152, 768]
        in_maps.append(
            {
                "xT": np.ascontiguousarray(x[b].T),
                "qkvT": np.ascontiguousarray(wsel.T),
                "maskb": maskb,
                "projT": np.ascontiguousarray(
                    proj_w[:, hg * HPC * D : (hg + 1) * HPC * D].T
                ).astype(np.float16),
            }
        )
    return in_maps


def kernel(x, local_attn_mask, qkv_w, proj_w, proj_b):
    proj_b = np.asarray(proj_b, dtype=np.float32)
    in_maps = _prepare_in_maps(x, local_attn_mask, qkv_w, proj_w, proj_b)

    if "nc" not in _CACHE:
        _CACHE["nc"] = _build_program()
    res = run_bass_kernel_spmd(
        _CACHE["nc"],
        in_maps,
        core_ids=list(range(NCORES)),
        tmpdir=os.environ.get("KPROF_DIR") or None,
    )
    _CACHE["last_result"] = res
    outs = res.results
    y = np.empty((B, N, C), dtype=np.float32)
    for b in range(B):
        y[b] = outs[2 * b]["out"] + outs[2 * b + 1]["out"] + proj_b[None, :]
    return y



# revision 27
# speedup vs baseline: 1.3643x; 1.0052x over previous
"""
Trainium2 Bass kernel for nn_Attention_29265907155069.

Reference computation (B=4, N=2048, C=768, H=12, D=64):
    qkv = x @ qkv_w.T -> split to q,k,v per head
    attn = softmax(q @ k.T * D + mask * -1e6)
    out  = (attn @ v) re-concat -> @ proj_w.T + proj_b

Sharding: 8 cores = (batch b in 0..3) x (head-group hg in 0..1, 6 heads each).
Each core computes its 6 heads' attention for its batch and a row-sharded
partial of the output projection; host sums the two head-group partials.

Per-core device pipeline:
  1. QKV: Q^T,K^T [d, n] and V [k, d] via PE matmuls (float32r). The D=64
     softmax scale is folded into Q on the host (Q-weights * 64).
  2. Scores S = 64*q@k.T per q-tile into PSUM (f32r), then an identity
     matmul accumulates -2^20 * mask (bf16, exact) onto the same PSUM tile.
  3. DVE reduce_max(negate) on PSUM -> -rowmax (masked entries sit at ~-1e6
     so the max is the masked row max).
  4. ACT: P = exp(S + (-rowmax)) from PSUM -> fp16 SBUF. Masked entries
     underflow to exactly 0, matching the reference's mask*-1e6 semantics.
  5. DMA xbar transpose P -> P^T (2-byte dtype, SBUF->SBUF).
  6. PV: O^T_unnorm[65, q] = [V | 1].T @ P^T accumulated over k tiles;
     row 64 = softmax denominators l (ones-column trick).
  7. recip(l) -> gpsimd partition_broadcast -> DVE mult => O^T normalized fp16.
  8. proj: Y[q, 768] = O^T.T @ projT (fp16) -> fp32 partial out.
"""

import os
import sys

import numpy as np

for _p in ("/opt/trn_rl_repo", "/root/.axon_site/_ro/trn_rl_repo"):
    if os.path.isdir(_p) and _p not in sys.path:
        sys.path.insert(0, _p)

import ml_dtypes  # noqa: E402

import concourse.mybir as mybir  # noqa: E402
from concourse import bacc  # noqa: E402
from concourse.bass_utils import run_bass_kernel_spmd  # noqa: E402
from concourse.masks import make_identity  # noqa: E402
from concourse.tile import TileContext  # noqa: E402

B, N, C, H = 4, 2048, 768, 12
D = C // H          # 64
HG = 2              # head groups (cores per batch)
HPC = H // HG       # heads per core = 6
CIN_T = C // 128    # 6 cin tiles
QT_TILES = 3        # 384 rows of Q^T (6 heads x 64) = 3 x 128
KT_TILES = N // 128  # 16
NCORES = 8
MASK_BIAS = -1048576.0  # -2^20, exact in bf16; scores are already x64

F32 = mybir.dt.float32
F32R = mybir.dt.float32r
F16 = mybir.dt.float16
BF16 = mybir.dt.bfloat16
F8E4 = mybir.dt.float8e4

# fp8 mask encoding (TRN2 f8e4 = IEEE e4m3, max +-240): ident diag = 128,
# mask entries = -128 -> product -16384 per masked score; scores are
# |s| <~ 3000 so masked entries sit at <= -13.5k, never the row max, and
# exp underflows to exactly 0.
IDENT_SCALE = 128.0
MASK_FP8_VAL = -128.0

_CACHE = {}


def _build_program(repeat=1):
    nc = bacc.Bacc(
        "TRN2",
        target_bir_lowering=False,
        debug=False,
        enable_asserts=False,
        num_devices=NCORES,
    )
    xT = nc.dram_tensor("xT", [C, N], F32R, kind="ExternalInput").ap()
    qkvT = nc.dram_tensor("qkvT", [C, 3 * HPC * D], F32R, kind="ExternalInput").ap()
    maskb = nc.dram_tensor("maskb", [N, N], F8E4, kind="ExternalInput").ap()
    projT = nc.dram_tensor("projT", [HPC * D, C], F16, kind="ExternalInput").ap()
    out = nc.dram_tensor("out", [N, C], F32, kind="ExternalOutput").ap()

    AL = mybir.AluOpType

    with TileContext(nc) as tc:
      for _rep in range(repeat):
        with tc.tile_pool(name="pers", bufs=1) as pers:
            # ---- persistent tiles ----
            QTs = [
                pers.tile([128, N], F32R, tag=f"qt{t}", name=f"qt{t}")
                for t in range(QT_TILES)
            ]
            KTs = [
                pers.tile([128, N], F32R, tag=f"kt{t}", name=f"kt{t}")
                for t in range(QT_TILES)
            ]
            # V augmented with a ones column: [128, (h,kt), 65]
            Vaug = pers.tile([128, HPC * KT_TILES, D + 1], F16, tag="vaug")
            Ocat = [
                pers.tile([128, N], F16, tag=f"oc{t}", name=f"oc{t}")
                for t in range(QT_TILES)
            ]
            PW = [
                pers.tile([128, C], F16, tag=f"pw{t}", name=f"pw{t}")
                for t in range(QT_TILES)
            ]
            identb = pers.tile([128, 128], BF16, tag="identb")
            ident = pers.tile([128, 128], F8E4, tag="ident")
            ones64 = pers.tile([1, D], F32, tag="ones64")

            make_identity(nc, identb[:, :])
            nc.scalar.mul(ident[:, :], identb[:, :], IDENT_SCALE)
            nc.vector.memset(ones64[:, :], 1.0)
            nc.gpsimd.memset(Vaug[:, :, D : D + 1], 1.0)
            for t in range(QT_TILES):
                nc.sync.dma_start(PW[t][:, :], projT[t * 128 : (t + 1) * 128, :])

            # ================= Phase 1: QKV projection =================
            with (
                tc.tile_pool(name="ph1", bufs=1) as p1,
                tc.tile_pool(name="ph1p", bufs=4, space="PSUM") as p1p,
            ):
                xts = [
                    p1.tile([128, N], F32R, tag=f"x{ci}", name=f"x{ci}")
                    for ci in range(CIN_T)
                ]
                wts = [
                    p1.tile([128, 3 * HPC * D], F32R, tag=f"w{ci}", name=f"w{ci}")
                    for ci in range(CIN_T)
                ]
                # load order: K weight cols + first x chunk first so the K
                # matmuls can start ~8us in instead of ~13us.
                koff = HPC * D
                for ci in range(CIN_T):
                    nc.scalar.dma_start(
                        wts[ci][:, koff : 2 * koff],
                        qkvT[ci * 128 : (ci + 1) * 128, koff : 2 * koff],
                    )
                    nc.sync.dma_start(
                        xts[ci][:, 0:512], xT[ci * 128 : (ci + 1) * 128, 0:512]
                    )
                for ci in range(CIN_T):
                    nc.scalar.dma_start(
                        wts[ci][:, 0:koff], qkvT[ci * 128 : (ci + 1) * 128, 0:koff]
                    )
                for qc in range(1, 4):
                    for ci in range(CIN_T):
                        nc.sync.dma_start(
                            xts[ci][:, qc * 512 : (qc + 1) * 512],
                            xT[ci * 128 : (ci + 1) * 128, qc * 512 : (qc + 1) * 512],
                        )
                for ci in range(CIN_T):
                    nc.scalar.dma_start(
                        wts[ci][:, 2 * koff : 3 * koff],
                        qkvT[ci * 128 : (ci + 1) * 128, 2 * koff : 3 * koff],
                    )

                # K^T then Q^T production: out[d_tile 128, q 512]
                for which, dst in ((1, KTs), (0, QTs)):
                    off = which * HPC * D  # 0 or 384 within qkvT cols
                    for qc in range(4):
                        for t in range(QT_TILES):
                            ps = p1p.tile([128, 512], F32, tag="p1ps", name="ps")
                            for ci in range(CIN_T):
                                nc.tensor.matmul(
                                    ps[:, :],
                                    wts[ci][:, off + t * 128 : off + (t + 1) * 128],
                                    xts[ci][:, qc * 512 : (qc + 1) * 512],
                                    start=(ci == 0),
                                    stop=(ci == CIN_T - 1),
                                )
                            nc.scalar.copy(
                                dst[t][:, qc * 512 : (qc + 1) * 512], ps[:, :]
                            )

                # V production: out[k_tile 128, 384] -> Vaug f16 (strided per head)
                voff = 2 * HPC * D  # 768
                for kt in range(KT_TILES):
                    ps = p1p.tile([128, HPC * D], F32, tag="p1ps", name="ps")
                    for ci in range(CIN_T):
                        nc.tensor.matmul(
                            ps[:, :],
                            xts[ci][:, kt * 128 : (kt + 1) * 128],
                            wts[ci][:, voff : voff + HPC * D],
                            start=(ci == 0),
                            stop=(ci == CIN_T - 1),
                        )
                    # psum [128, (h 6, d 64)] -> Vaug[:, h*16+kt, 0:64]
                    nc.scalar.copy(
                        Vaug[:, kt :: KT_TILES, 0:D],
                        ps[:, :].rearrange("p (h d) -> p h d", h=HPC),
                    )

            # ================= Phase 2: attention =================
            # Software-pipelined emission: PV groups lag the score/softmax
            # stream by PV_LAG score-groups so the DVE always has reduce work
            # queued while the PE chews through PV/proj matmuls.
            PV_LAG = 3
            with (
                tc.tile_pool(name="mk", bufs=2) as pmk,
                tc.tile_pool(name="work", bufs=2) as pw,
                tc.tile_pool(name="psS", bufs=3, space="PSUM") as psS,
                tc.tile_pool(name="psO", bufs=2, space="PSUM") as psO,
            ):
                mks = {}     # qc -> list of 4 mask tiles
                PT = {}      # (qc, hp) -> [PT0, PT1]

                def load_masks(qc):
                    # scalar-engine HWDGE queue: keeps mask loads from
                    # head-of-line blocking the transposes on the sync queue
                    tiles = []
                    for j in range(4):
                        mk = pmk.tile([128, N], F8E4, tag=f"mk{j}", name=f"mk{j}")
                        row0 = qc * 512 + j * 128
                        nc.sync.dma_start(mk[:, :], maskb[row0 : row0 + 128, :])
                        tiles.append(mk)
                    mks[qc] = tiles

                def score_group(qc, hp, j):
                    if j == 0:
                        PT[(qc, hp)] = [
                            pw.tile(
                                [128, 4, KT_TILES, 128], F16, tag="ptrans",
                                name=f"PT{a}", bufs=4,
                            )
                            for a in range(2)
                        ]
                    PTs = PT[(qc, hp)]
                    qt = qc * 4 + j
                    mstats = pw.tile([128, 4], F32, tag="mstat", name="mstat",
                                     bufs=6)
                    negm = pw.tile([128, 2], F32, tag="negm", name="negm",
                                   bufs=6)
                    fs = pw.tile([128, 2], F32, tag="fs", name="fs", bufs=6)
                    pns = [
                        pw.tile([128, N], F16, tag=f"pn{a}", name=f"pn{a}",
                                bufs=3)
                        for a in range(2)
                    ]
                    for half in range(2):
                        sps = [
                            psS.tile([128, 1024], F32, tag="sp", name=f"sp{a}")
                            for a in range(2)
                        ]
                        # packed K=64 score matmuls: head a in row group a
                        for c in range(2):
                            kc = half * 1024 + c * 512
                            for a in range(2):
                                nc.tensor.matmul(
                                    sps[a][:, c * 512 : (c + 1) * 512],
                                    QTs[hp][
                                        a * D : (a + 1) * D,
                                        qt * 128 : (qt + 1) * 128,
                                    ],
                                    KTs[hp][a * D : (a + 1) * D, kc : kc + 512],
                                    start=True,
                                    stop=False,
                                    tile_position=(a * D, 0),
                                )
                        for c in range(2):
                            kc = half * 1024 + c * 512
                            for a in range(2):
                                nc.tensor.matmul(
                                    sps[a][:, c * 512 : (c + 1) * 512],
                                    ident[:, :],
                                    mks[qc][j][:, kc : kc + 512],
                                    start=False,
                                    stop=True,
                                )
                        for a in range(2):
                            # mstats layout: [a0h0, a0h1, a1h0, a1h1]
                            col = a * 2 + half
                            nc.vector.tensor_reduce(
                                mstats[:, col : col + 1],
                                sps[a][:, :],
                                axis=mybir.AxisListType.X,
                                op=AL.max,
                                negate=True,
                            )
                            if half == 0:
                                # early exp with the half-0 max; fs0
                                # correction lands after negm is known
                                nc.scalar.activation(
                                    pns[a][:, 0:1024],
                                    sps[a][:, :],
                                    mybir.ActivationFunctionType.Exp,
                                    bias=mstats[:, 2 * a : 2 * a + 1],
                                    scale=1.0,
                                )
                        if half == 1:
                            for a in range(2):
                                nc.vector.tensor_reduce(
                                    negm[:, a : a + 1],
                                    mstats[:, 2 * a : 2 * a + 2],
                                    axis=mybir.AxisListType.X,
                                    op=AL.min,
                                )
                                # fs0 = exp(m0 - m) first so the half-0
                                # rescale isn't queued behind the big exp
                                nc.scalar.activation(
                                    fs[:, a : a + 1],
                                    mstats[:, 2 * a : 2 * a + 1],
                                    mybir.ActivationFunctionType.Exp,
                                    bias=negm[:, a : a + 1],
                                    scale=-1.0,
                                )
                                nc.scalar.activation(
                                    pns[a][:, 1024:2048],
                                    sps[a][:, :],
                                    mybir.ActivationFunctionType.Exp,
                                    bias=negm[:, a : a + 1],
                                    scale=1.0,
                                )
                                nc.vector.tensor_scalar(
                                    pns[a][:, 0:1024],
                                    pns[a][:, 0:1024],
                                    fs[:, a : a + 1],
                                    None,
                                    op0=AL.mult,
                                )
                                nc.sync.dma_start_transpose(
                                    PTs[a][:, j, 0:8, :],
                                    pns[a][:, 0:1024],
                                )
                                nc.sync.dma_start_transpose(
                                    PTs[a][:, j, 8:16, :],
                                    pns[a][:, 1024:2048],
                                )

                def pv_group(qc, hp, a):
                    PTs = PT[(qc, hp)]
                    h = 2 * hp + a
                    ht, hpp = hp, a * D
                    # PV: O^T_unnorm [65, 512q]
                    ot = psO.tile([D + 1, 512], F32, tag="ot", name="ot")
                    for kt in range(KT_TILES):
                        nc.tensor.matmul(
                            ot[:, :],
                            Vaug[:, h * KT_TILES + kt, :],
                            PTs[a][:, :, kt, :],
                            start=(kt == 0),
                            stop=(kt == KT_TILES - 1),
                        )
                    rl = pw.tile([1, 512], F32, tag="rl", name="rl", bufs=4)
                    nc.vector.reciprocal(rl[:, :], ot[D : D + 1, :])
                    rb = pw.tile([D, 512], F32, tag="rb", name="rb", bufs=4)
                    nc.gpsimd.partition_broadcast(rb[:, :], rl[:, :])
                    nc.vector.tensor_tensor(
                        Ocat[ht][hpp : hpp + D, qc * 512 : (qc + 1) * 512],
                        ot[0:D, :],
                        rb[:, :],
                        op=AL.mult,
                    )
                    if a == 1:
                        PT.pop((qc, hp))

                def proj_group(qc, j):
                    qt = qc * 4 + j
                    y0 = psO.tile([128, 512], F32, tag="ot", name="y0")
                    y1 = psO.tile([128, 256], F32, tag="ot", name="y1")
                    for ct in range(QT_TILES):
                        lt = Ocat[ct][:, qt * 128 : (qt + 1) * 128]
                        nc.tensor.matmul(
                            y0[:, :], lt, PW[ct][:, 0:512],
                            start=(ct == 0), stop=(ct == QT_TILES - 1),
                        )
                        nc.tensor.matmul(
                            y1[:, :], lt, PW[ct][:, 512:768],
                            start=(ct == 0), stop=(ct == QT_TILES - 1),
                        )
                    ysb = pw.tile([128, C], F32, tag="ysb", name="ysb")
                    nc.scalar.copy(ysb[:, 0:512], y0[:, :])
                    nc.scalar.copy(ysb[:, 512:768], y1[:, :])
                    nc.sync.dma_start(out[qt * 128 : (qt + 1) * 128, :], ysb[:, :])

                # flat schedule: score groups in (qc, hp, j) order; each
                # deferred task fires PV_LAG group-slots after its data is
                # complete.
                groups = [
                    (qc, hp, j)
                    for qc in range(4)
                    for hp in range(QT_TILES)
                    for j in range(4)
                ]
                pending = []  # (due_slot, fn)
                load_masks(0)
                for slot, (qc, hp, j) in enumerate(groups):
                    if j == 0 and hp == 1 and qc + 1 < 4:
                        load_masks(qc + 1)  # prefetch next q-chunk's masks
                    for due, fn in [p for p in pending if p[0] <= slot]:
                        fn()
                        pending.remove((due, fn))
                    score_group(qc, hp, j)
                    if j == 3:
                        # the last head-pair's PV can lag further: its PT ring
                        # slot isn't needed until deep into the next q-chunk
                        lag = PV_LAG
                        for a in range(2):
                            pending.append(
                                (
                                    slot + lag + a,
                                    lambda qc=qc, hp=hp, a=a: pv_group(qc, hp, a),
                                )
                            )
                        if hp == QT_TILES - 1:
                            for jj in range(4):
                                pending.append(
                                    (
                                        slot + lag + 2 + jj,
                                        lambda qc=qc, jj=jj: proj_group(qc, jj),
                                    )
                                )
                for due, fn in sorted(pending, key=lambda p: p[0]):
                    fn()
    nc.compile()
    return nc


def _prepare_in_maps(x, local_attn_mask, qkv_w, proj_w, proj_b):
    x = np.asarray(x, dtype=np.float32)
    mask = np.asarray(local_attn_mask)
    qkv_w = np.asarray(qkv_w, dtype=np.float32)
    proj_w = np.asarray(proj_w, dtype=np.float32)

    maskb = (MASK_FP8_VAL * mask.astype(np.float32)).astype(ml_dtypes.float8_e4m3)
    in_maps = []
    for c in range(NCORES):
        b, hg = c // HG, c % HG
        rq = slice(hg * HPC * D, (hg + 1) * HPC * D)
        rk = slice(C + hg * HPC * D, C + (hg + 1) * HPC * D)
        rv = slice(2 * C + hg * HPC * D, 2 * C + (hg + 1) * HPC * D)
        # softmax scale D folded into the Q weights
        wsel = np.concatenate(
            [qkv_w[rq] * float(D), qkv_w[rk], qkv_w[rv]], axis=0
        )  # [1152, 768]
        in_maps.append(
            {
                "xT": np.ascontiguousarray(x[b].T),
                "qkvT": np.ascontiguousarray(wsel.T),
                "maskb": maskb,
                "projT": np.ascontiguousarray(
                    proj_w[:, hg * HPC * D : (hg + 1) * HPC * D].T
                ).astype(np.float16),
            }
        )
    return in_maps


def kernel(x, local_attn_mask, qkv_w, proj_w, proj_b):
    proj_b = np.asarray(proj_b, dtype=np.float32)
    in_maps = _prepare_in_maps(x, local_attn_mask, qkv_w, proj_w, proj_b)

    if "nc" not in _CACHE:
        _CACHE["nc"] = _build_program()
    res = run_bass_kernel_spmd(
        _CACHE["nc"],
        in_maps,
        core_ids=list(range(NCORES)),
        tmpdir=os.environ.get("KPROF_DIR") or None,
    )
    _CACHE["last_result"] = res
    outs = res.results
    y = np.empty((B, N, C), dtype=np.float32)
    for b in range(B):
        y[b] = outs[2 * b]["out"] + outs[2 * b + 1]["out"] + proj_b[None, :]
    return y



# revision 30
# speedup vs baseline: 164.4523x; 120.5391x over previous
"""
Trainium2 Bass kernel for nn_Attention_29265907155069.

Reference computation (B=4, N=2048, C=768, H=12, D=64):
    qkv = x @ qkv_w.T -> split to q,k,v per head
    attn = softmax(q @ k.T * D + mask * -1e6)
    out  = (attn @ v) re-concat -> @ proj_w.T + proj_b

Sharding: 8 cores = (batch b in 0..3) x (head-group hg in 0..1, 6 heads each).
Each core computes its 6 heads' attention for its batch and a row-sharded
partial of the output projection; host sums the two head-group partials.

Per-core device pipeline:
  1. QKV: Q^T,K^T [d, n] and V [k, d] via PE matmuls (float32r; full rate at
     free-dim >= 256). The D=64 softmax scale is folded into Q on the host.
     K-weight cols + first x chunk DMA first so the PE starts ~8us in.
  2. Scores S = 64*q@k.T per q-tile into PSUM (f32r, two heads packed into
     PE row-groups via tile_position), then an fp8 identity matmul (diag=128)
     accumulates -16384 * mask (mask entries -128 in f8e4/e4m3) onto the same
     PSUM tile. Masked entries sit at ~-1.4e4: never the row max, exp -> 0.
  3. DVE reduce_max(negate) per 1024-wide half -> -m_half.
  4. Asymmetric exp (ACT): half 0 exps early with bias -m0 and is rescaled
     by fs0 = exp(m0 - m) once m = max(m0, m1) is known (one DVE
     tensor_scalar per head instead of two); half 1 exps once with the
     final bias -m. P is fp16; masked entries are exactly 0.
  5. DMA xbar transpose P -> P^T, split per half (the half-0 part leaves as
     soon as its rescale lands). P^T layout [128k, (j, kt, 128q)] makes each
     per-j transpose a contiguous 4KB/partition write (fast xbar path).
  6. PV: O^T_unnorm[65, q] = [V | 1].T @ P^T accumulated over k tiles;
     row 64 = softmax denominators l (ones-column trick).
  7. recip(l) -> gpsimd partition_broadcast -> DVE mult => O^T normalized fp16.
     (NOTE: nc.vector.reciprocal_approx_fast silently corrupts results on HW
     for this [1,512] AP — keep the exact reciprocal.)
  8. proj: Y[q, 768] = O^T.T @ projT (fp16) -> fp32 partial out.

Emission is software-pipelined: PV groups lag the score/softmax stream by
PV_LAG=3 score-groups and projs trail further, so the DVE (the pacing
engine: ~235us of irreducible 1x PSUM reduce_max) stays fed while the PE
runs PV/proj matmul bursts. TimelineSim (warm-PE model): 436us/core vs
502us for the unpipelined baseline; measured HW steady-state ~386us/iter.
"""

import os
import sys

import numpy as np

for _p in ("/opt/trn_rl_repo", "/root/.axon_site/_ro/trn_rl_repo"):
    if os.path.isdir(_p) and _p not in sys.path:
        sys.path.insert(0, _p)

import ml_dtypes  # noqa: E402

import concourse.mybir as mybir  # noqa: E402
from concourse import bacc  # noqa: E402
from concourse.bass_utils import run_bass_kernel_spmd  # noqa: E402
from concourse.masks import make_identity  # noqa: E402
from concourse.tile import TileContext  # noqa: E402

B, N, C, H = 4, 2048, 768, 12
D = C // H          # 64
HG = 2              # head groups (cores per batch)
HPC = H // HG       # heads per core = 6
CIN_T = C // 128    # 6 cin tiles
QT_TILES = 3        # 384 rows of Q^T (6 heads x 64) = 3 x 128
KT_TILES = N // 128  # 16
NCORES = 8
MASK_BIAS = -1048576.0  # -2^20, exact in bf16; scores are already x64

F32 = mybir.dt.float32
F32R = mybir.dt.float32r
F16 = mybir.dt.float16
BF16 = mybir.dt.bfloat16
F8E4 = mybir.dt.float8e4

# fp8 mask encoding (TRN2 f8e4 = IEEE e4m3, max +-240): ident diag = 128,
# mask entries = -128 -> product -16384 per masked score; scores are
# |s| <~ 3000 so masked entries sit at <= -13.5k, never the row max, and
# exp underflows to exactly 0.
IDENT_SCALE = 128.0
MASK_FP8_VAL = -128.0

_CACHE = {}


def _build_program(repeat=1):
    nc = bacc.Bacc(
        "TRN2",
        target_bir_lowering=False,
        debug=False,
        enable_asserts=False,
        num_devices=NCORES,
    )
    xT = nc.dram_tensor("xT", [C, N], F32R, kind="ExternalInput").ap()
    qkvT = nc.dram_tensor("qkvT", [C, 3 * HPC * D], F32R, kind="ExternalInput").ap()
    maskb = nc.dram_tensor("maskb", [N, N], F8E4, kind="ExternalInput").ap()
    projT = nc.dram_tensor("projT", [HPC * D, C], F16, kind="ExternalInput").ap()
    out = nc.dram_tensor("out", [N, C], F32, kind="ExternalOutput").ap()

    AL = mybir.AluOpType

    with TileContext(nc) as tc:
      for _rep in range(repeat):
        with tc.tile_pool(name="pers", bufs=1) as pers:
            # ---- persistent tiles ----
            QTs = [
                pers.tile([128, N], F32R, tag=f"qt{t}", name=f"qt{t}")
                for t in range(QT_TILES)
            ]
            KTs = [
                pers.tile([128, N], F32R, tag=f"kt{t}", name=f"kt{t}")
                for t in range(QT_TILES)
            ]
            # V augmented with a ones column: [128, (h,kt), 65]
            Vaug = pers.tile([128, HPC * KT_TILES, D + 1], F16, tag="vaug")
            Ocat = [
                pers.tile([128, N], F16, tag=f"oc{t}", name=f"oc{t}")
                for t in range(QT_TILES)
            ]
            PW = [
                pers.tile([128, C], F16, tag=f"pw{t}", name=f"pw{t}")
                for t in range(QT_TILES)
            ]
            identb = pers.tile([128, 128], BF16, tag="identb")
            ident = pers.tile([128, 128], F8E4, tag="ident")
            ones64 = pers.tile([1, D], F32, tag="ones64")

            make_identity(nc, identb[:, :])
            nc.scalar.mul(ident[:, :], identb[:, :], IDENT_SCALE)
            nc.vector.memset(ones64[:, :], 1.0)
            nc.gpsimd.memset(Vaug[:, :, D : D + 1], 1.0)
            for t in range(QT_TILES):
                nc.sync.dma_start(PW[t][:, :], projT[t * 128 : (t + 1) * 128, :])

            # ================= Phase 1: QKV projection =================
            with (
                tc.tile_pool(name="ph1", bufs=1) as p1,
                tc.tile_pool(name="ph1p", bufs=4, space="PSUM") as p1p,
            ):
                xts = [
                    p1.tile([128, N], F32R, tag=f"x{ci}", name=f"x{ci}")
                    for ci in range(CIN_T)
                ]
                wts = [
                    p1.tile([128, 3 * HPC * D], F32R, tag=f"w{ci}", name=f"w{ci}")
                    for ci in range(CIN_T)
                ]
                # load order: K weight cols + first x chunk first so the K
                # matmuls can start ~8us in instead of ~13us.
                koff = HPC * D
                for ci in range(CIN_T):
                    nc.scalar.dma_start(
                        wts[ci][:, koff : 2 * koff],
                        qkvT[ci * 128 : (ci + 1) * 128, koff : 2 * koff],
                    )
                    nc.sync.dma_start(
                        xts[ci][:, 0:512], xT[ci * 128 : (ci + 1) * 128, 0:512]
                    )
                for ci in range(CIN_T):
                    nc.scalar.dma_start(
                        wts[ci][:, 0:koff], qkvT[ci * 128 : (ci + 1) * 128, 0:koff]
                    )
                for qc in range(1, 4):
                    for ci in range(CIN_T):
                        nc.sync.dma_start(
                            xts[ci][:, qc * 512 : (qc + 1) * 512],
                            xT[ci * 128 : (ci + 1) * 128, qc * 512 : (qc + 1) * 512],
                        )
                for ci in range(CIN_T):
                    nc.scalar.dma_start(
                        wts[ci][:, 2 * koff : 3 * koff],
                        qkvT[ci * 128 : (ci + 1) * 128, 2 * koff : 3 * koff],
                    )

                # K^T then Q^T production: out[d_tile 128, q 512]
                for which, dst in ((1, KTs), (0, QTs)):
                    off = which * HPC * D  # 0 or 384 within qkvT cols
                    for qc in range(4):
                        for t in range(QT_TILES):
                            ps = p1p.tile([128, 512], F32, tag="p1ps", name="ps")
                            for ci in range(CIN_T):
                                nc.tensor.matmul(
                                    ps[:, :],
                                    wts[ci][:, off + t * 128 : off + (t + 1) * 128],
                                    xts[ci][:, qc * 512 : (qc + 1) * 512],
                                    start=(ci == 0),
                                    stop=(ci == CIN_T - 1),
                                )
                            nc.scalar.copy(
                                dst[t][:, qc * 512 : (qc + 1) * 512], ps[:, :]
                            )

                # V production: out[k_tile 128, 384] -> Vaug f16 (strided per head)
                voff = 2 * HPC * D  # 768
                for kt in range(KT_TILES):
                    ps = p1p.tile([128, HPC * D], F32, tag="p1ps", name="ps")
                    for ci in range(CIN_T):
                        nc.tensor.matmul(
                            ps[:, :],
                            xts[ci][:, kt * 128 : (kt + 1) * 128],
                            wts[ci][:, voff : voff + HPC * D],
                            start=(ci == 0),
                            stop=(ci == CIN_T - 1),
                        )
                    # psum [128, (h 6, d 64)] -> Vaug[:, h*16+kt, 0:64]
                    nc.scalar.copy(
                        Vaug[:, kt :: KT_TILES, 0:D],
                        ps[:, :].rearrange("p (h d) -> p h d", h=HPC),
                    )

            # ================= Phase 2: attention =================
            # Software-pipelined emission: PV groups lag the score/softmax
            # stream by PV_LAG score-groups so the DVE always has reduce work
            # queued while the PE chews through PV/proj matmuls.
            PV_LAG = 3
            with (
                tc.tile_pool(name="mk", bufs=2) as pmk,
                tc.tile_pool(name="work", bufs=2) as pw,
                tc.tile_pool(name="psS", bufs=3, space="PSUM") as psS,
                tc.tile_pool(name="psO", bufs=2, space="PSUM") as psO,
            ):
                mks = {}     # qc -> list of 4 mask tiles
                PT = {}      # (qc, hp) -> [PT0, PT1]

                def load_masks(qc):
                    # scalar-engine HWDGE queue: keeps mask loads from
                    # head-of-line blocking the transposes on the sync queue
                    tiles = []
                    for j in range(4):
                        mk = pmk.tile([128, N], F8E4, tag=f"mk{j}", name=f"mk{j}")
                        row0 = qc * 512 + j * 128
                        nc.sync.dma_start(mk[:, :], maskb[row0 : row0 + 128, :])
                        tiles.append(mk)
                    mks[qc] = tiles

                def score_group(qc, hp, j):
                    if j == 0:
                        PT[(qc, hp)] = [
                            pw.tile(
                                [128, 4, KT_TILES, 128], F16, tag="ptrans",
                                name=f"PT{a}", bufs=4,
                            )
                            for a in range(2)
                        ]
                    PTs = PT[(qc, hp)]
                    qt = qc * 4 + j
                    mstats = pw.tile([128, 4], F32, tag="mstat", name="mstat",
                                     bufs=6)
                    negm = pw.tile([128, 2], F32, tag="negm", name="negm",
                                   bufs=6)
                    fs = pw.tile([128, 2], F32, tag="fs", name="fs", bufs=6)
                    pns = [
                        pw.tile([128, N], F16, tag=f"pn{a}", name=f"pn{a}",
                                bufs=3)
                        for a in range(2)
                    ]
                    for half in range(2):
                        sps = [
                            psS.tile([128, 1024], F32, tag="sp", name=f"sp{a}")
                            for a in range(2)
                        ]
                        # packed K=64 score matmuls: head a in row group a
                        for c in range(2):
                            kc = half * 1024 + c * 512
                            for a in range(2):
                                nc.tensor.matmul(
                                    sps[a][:, c * 512 : (c + 1) * 512],
                                    QTs[hp][
                                        a * D : (a + 1) * D,
                                        qt * 128 : (qt + 1) * 128,
                                    ],
                                    KTs[hp][a * D : (a + 1) * D, kc : kc + 512],
                                    start=True,
                                    stop=False,
                                    tile_position=(a * D, 0),
                                )
                        for c in range(2):
                            kc = half * 1024 + c * 512
                            for a in range(2):
                                nc.tensor.matmul(
                                    sps[a][:, c * 512 : (c + 1) * 512],
                                    ident[:, :],
                                    mks[qc][j][:, kc : kc + 512],
                                    start=False,
                                    stop=True,
                                )
                        for a in range(2):
                            # mstats layout: [a0h0, a0h1, a1h0, a1h1]
                            col = a * 2 + half
                            nc.vector.tensor_reduce(
                                mstats[:, col : col + 1],
                                sps[a][:, :],
                                axis=mybir.AxisListType.X,
                                op=AL.max,
                                negate=True,
                            )
                            if half == 0:
                                # early exp with the half-0 max; fs0
                                # correction lands after negm is known
                                nc.scalar.activation(
                                    pns[a][:, 0:1024],
                                    sps[a][:, :],
                                    mybir.ActivationFunctionType.Exp,
                                    bias=mstats[:, 2 * a : 2 * a + 1],
                                    scale=1.0,
                                )
                        if half == 1:
                            for a in range(2):
                                nc.vector.tensor_reduce(
                                    negm[:, a : a + 1],
                                    mstats[:, 2 * a : 2 * a + 2],
                                    axis=mybir.AxisListType.X,
                                    op=AL.min,
                                )
                                # fs0 = exp(m0 - m) first so the half-0
                                # rescale isn't queued behind the big exp
                                nc.scalar.activation(
                                    fs[:, a : a + 1],
                                    mstats[:, 2 * a : 2 * a + 1],
                                    mybir.ActivationFunctionType.Exp,
                                    bias=negm[:, a : a + 1],
                                    scale=-1.0,
                                )
                                nc.scalar.activation(
                                    pns[a][:, 1024:2048],
                                    sps[a][:, :],
                                    mybir.ActivationFunctionType.Exp,
                                    bias=negm[:, a : a + 1],
                                    scale=1.0,
                                )
                                nc.vector.tensor_scalar(
                                    pns[a][:, 0:1024],
                                    pns[a][:, 0:1024],
                                    fs[:, a : a + 1],
                                    None,
                                    op0=AL.mult,
                                )
                                nc.sync.dma_start_transpose(
                                    PTs[a][:, j, 0:8, :],
                                    pns[a][:, 0:1024],
                                )
                                nc.sync.dma_start_transpose(
                                    PTs[a][:, j, 8:16, :],
                                    pns[a][:, 1024:2048],
                                )

                def pv_group(qc, hp, a):
                    PTs = PT[(qc, hp)]
                    h = 2 * hp + a
                    ht, hpp = hp, a * D
                    # PV: O^T_unnorm [65, 512q]
                    ot = psO.tile([D + 1, 512], F32, tag="ot", name="ot")
                    for kt in range(KT_TILES):
                        nc.tensor.matmul(
                            ot[:, :],
                            Vaug[:, h * KT_TILES + kt, :],
                            PTs[a][:, :, kt, :],
                            start=(kt == 0),
                            stop=(kt == KT_TILES - 1),
                        )
                    rl = pw.tile([1, 512], F32, tag="rl", name="rl", bufs=4)
                    nc.vector.reciprocal(rl[:, :], ot[D : D + 1, :])
                    rb = pw.tile([D, 512], F32, tag="rb", name="rb", bufs=4)
                    nc.gpsimd.partition_broadcast(rb[:, :], rl[:, :])
                    nc.vector.tensor_tensor(
                        Ocat[ht][hpp : hpp + D, qc * 512 : (qc + 1) * 512],
                        ot[0:D, :],
                        rb[:, :],
                        op=AL.mult,
                    )
                    if a == 1:
                        PT.pop((qc, hp))

                def proj_group(qc, j):
                    qt = qc * 4 + j
                    y0 = psO.tile([128, 512], F32, tag="ot", name="y0")
                    y1 = psO.tile([128, 256], F32, tag="ot", name="y1")
                    for ct in range(QT_TILES):
                        lt = Ocat[ct][:, qt * 128 : (qt + 1) * 128]
                        nc.tensor.matmul(
                            y0[:, :], lt, PW[ct][:, 0:512],
                            start=(ct == 0), stop=(ct == QT_TILES - 1),
                        )
                        nc.tensor.matmul(
                            y1[:, :], lt, PW[ct][:, 512:768],
                            start=(ct == 0), stop=(ct == QT_TILES - 1),
                        )
                    ysb = pw.tile([128, C], F32, tag="ysb", name="ysb")
                    nc.scalar.copy(ysb[:, 0:512], y0[:, :])
                    nc.scalar.copy(ysb[:, 512:768], y1[:, :])
                    nc.sync.dma_start(out[qt * 128 : (qt + 1) * 128, :], ysb[:, :])

                # flat schedule: score groups in (qc, hp, j) order; each
                # deferred task fires PV_LAG group-slots after its data is
                # complete.
                groups = [
                    (qc, hp, j)
                    for qc in range(4)
                    for hp in range(QT_TILES)
                    for j in range(4)
                ]
                pending = []  # (due_slot, fn)
                load_masks(0)
                for slot, (qc, hp, j) in enumerate(groups):
                    if j == 0 and hp == 1 and qc + 1 < 4:
                        load_masks(qc + 1)  # prefetch next q-chunk's masks
                    for due, fn in [p for p in pending if p[0] <= slot]:
                        fn()
                        pending.remove((due, fn))
                    score_group(qc, hp, j)
                    if j == 3:
                        # the last head-pair's PV can lag further: its PT ring
                        # slot isn't needed until deep into the next q-chunk
                        lag = PV_LAG
                        for a in range(2):
                            pending.append(
                                (
                                    slot + lag + a,
                                    lambda qc=qc, hp=hp, a=a: pv_group(qc, hp, a),
                                )
                            )
                        if hp == QT_TILES - 1:
                            for jj in range(4):
                                pending.append(
                                    (
                                        slot + lag + 2 + jj,
                                        lambda qc=qc, jj=jj: proj_group(qc, jj),
                                    )
                                )
                for due, fn in sorted(pending, key=lambda p: p[0]):
                    fn()
    nc.compile()
    return nc


def _prepare_in_maps(x, local_attn_mask, qkv_w, proj_w, proj_b):
    x = np.asarray(x, dtype=np.float32)
    mask = np.asarray(local_attn_mask)
    qkv_w = np.asarray(qkv_w, dtype=np.float32)
    proj_w = np.asarray(proj_w, dtype=np.float32)

    maskb = (MASK_FP8_VAL * mask.astype(np.float32)).astype(ml_dtypes.float8_e4m3)
    in_maps = []
    for c in range(NCORES):
        b, hg = c // HG, c % HG
        rq = slice(hg * HPC * D, (hg + 1) * HPC * D)
        rk = slice(C + hg * HPC * D, C + (hg + 1) * HPC * D)
        rv = slice(2 * C + hg * HPC * D, 2 * C + (hg + 1) * HPC * D)
        # softmax scale D folded into the Q weights
        wsel = np.concatenate(
            [qkv_w[rq] * float(D), qkv_w[rk], qkv_w[rv]], axis=0
        )  # [1152, 768]
        in_maps.append(
            {
                "xT": np.ascontiguousarray(x[b].T),
                "qkvT": np.ascontiguousarray(wsel.T),
                "maskb": maskb,
                "projT": np.ascontiguousarray(
                    proj_w[:, hg * HPC * D : (hg + 1) * HPC * D].T
                ).astype(np.float16),
            }
        )
    return in_maps


def kernel(x, local_attn_mask, qkv_w, proj_w, proj_b):
    proj_b = np.asarray(proj_b, dtype=np.float32)
    in_maps = _prepare_in_maps(x, local_attn_mask, qkv_w, proj_w, proj_b)

    if "nc" not in _CACHE:
        _CACHE["nc"] = _build_program()
    res = run_bass_kernel_spmd(
        _CACHE["nc"],
        in_maps,
        core_ids=list(range(NCORES)),
        tmpdir=os.environ.get("KPROF_DIR") or None,
    )
    _CACHE["last_result"] = res
    outs = res.results
    y = np.empty((B, N, C), dtype=np.float32)
    for b in range(B):
        y[b] = outs[2 * b]["out"] + outs[2 * b + 1]["out"] + proj_b[None, :]
    return y



# revision 32
# speedup vs baseline: 264.8486x; 1.6105x over previous
"""
Trainium2 Bass kernel for nn_Attention_29265907155069.

Reference computation (B=4, N=2048, C=768, H=12, D=64):
    qkv = x @ qkv_w.T -> split to q,k,v per head
    attn = softmax(q @ k.T * D + mask * -1e6)
    out  = (attn @ v) re-concat -> @ proj_w.T + proj_b

Sharding: 8 cores = (batch b in 0..3) x (head-group hg in 0..1, 6 heads each).
Each core computes its 6 heads' attention for its batch and a row-sharded
partial of the output projection; host sums the two head-group partials.

Per-core device pipeline:
  1. QKV: Q^T,K^T [d, n] and V [k, d] via PE matmuls (float32r). The D=64
     softmax scale is folded into Q on the host (Q-weights * 64).
  2. Scores S = 64*q@k.T per q-tile into PSUM (f32r), then an identity
     matmul accumulates -2^20 * mask (bf16, exact) onto the same PSUM tile.
  3. DVE reduce_max(negate) on PSUM -> -rowmax (masked entries sit at ~-1e6
     so the max is the masked row max).
  4. ACT: P = exp(S + (-rowmax)) from PSUM -> fp16 SBUF. Masked entries
     underflow to exactly 0, matching the reference's mask*-1e6 semantics.
  5. DMA xbar transpose P -> P^T (2-byte dtype, SBUF->SBUF).
  6. PV: O^T_unnorm[65, q] = [V | 1].T @ P^T accumulated over k tiles;
     row 64 = softmax denominators l (ones-column trick).
  7. recip(l) -> gpsimd partition_broadcast -> DVE mult => O^T normalized fp16.
  8. proj: Y[q, 768] = O^T.T @ projT (fp16) -> fp32 partial out.
"""

import os
import sys

import numpy as np

for _p in ("/opt/trn_rl_repo", "/root/.axon_site/_ro/trn_rl_repo"):
    if os.path.isdir(_p) and _p not in sys.path:
        sys.path.insert(0, _p)

import ml_dtypes  # noqa: E402

import concourse.mybir as mybir  # noqa: E402
from concourse import bacc  # noqa: E402
from concourse.bass_utils import run_bass_kernel_spmd  # noqa: E402
from concourse.masks import make_identity  # noqa: E402
from concourse.tile import TileContext  # noqa: E402

B, N, C, H = 4, 2048, 768, 12
D = C // H          # 64
HG = 2              # head groups (cores per batch)
HPC = H // HG       # heads per core = 6
CIN_T = C // 128    # 6 cin tiles
QT_TILES = 3        # 384 rows of Q^T (6 heads x 64) = 3 x 128
KT_TILES = N // 128  # 16
NCORES = 8
MASK_BIAS = -1048576.0  # -2^20, exact in bf16; scores are already x64

F32 = mybir.dt.float32
F32R = mybir.dt.float32r
F16 = mybir.dt.float16
BF16 = mybir.dt.bfloat16
F8E4 = mybir.dt.float8e4

# fp8 mask encoding (TRN2 f8e4 = IEEE e4m3, max +-240): ident diag = 128,
# mask entries = -128 -> product -16384 per masked score; scores are
# |s| <~ 3000 so masked entries sit at <= -13.5k, never the row max, and
# exp underflows to exactly 0.
IDENT_SCALE = 128.0
MASK_FP8_VAL = -128.0

_CACHE = {}


def _build_program(repeat=1):
    nc = bacc.Bacc(
        "TRN2",
        target_bir_lowering=False,
        debug=False,
        enable_asserts=False,
        num_devices=NCORES,
    )
    xT = nc.dram_tensor("xT", [C, N], F32R, kind="ExternalInput").ap()
    qkvT = nc.dram_tensor("qkvT", [C, 3 * HPC * D], F32R, kind="ExternalInput").ap()
    maskb = nc.dram_tensor("maskb", [N, N], F8E4, kind="ExternalInput").ap()
    projT = nc.dram_tensor("projT", [HPC * D, C], F16, kind="ExternalInput").ap()
    out = nc.dram_tensor("out", [N, C], F32, kind="ExternalOutput").ap()

    AL = mybir.AluOpType

    with TileContext(nc) as tc:
      for _rep in range(repeat):
        with tc.tile_pool(name="pers", bufs=1) as pers:
            # ---- persistent tiles ----
            QTs = [
                pers.tile([128, N], F32R, tag=f"qt{t}", name=f"qt{t}")
                for t in range(QT_TILES)
            ]
            KTs = [
                pers.tile([128, N], F32R, tag=f"kt{t}", name=f"kt{t}")
                for t in range(QT_TILES)
            ]
            # V augmented with a ones column: [128, (h,kt), 65]
            Vaug = pers.tile([128, HPC * KT_TILES, D + 1], F16, tag="vaug")
            Ocat = [
                pers.tile([128, N], F16, tag=f"oc{t}", name=f"oc{t}")
                for t in range(QT_TILES)
            ]
            PW = [
                pers.tile([128, C], F16, tag=f"pw{t}", name=f"pw{t}")
                for t in range(QT_TILES)
            ]
            identb = pers.tile([128, 128], BF16, tag="identb")
            ident = pers.tile([128, 128], F8E4, tag="ident")
            ones64 = pers.tile([1, D], F32, tag="ones64")

            make_identity(nc, identb[:, :])
            nc.scalar.mul(ident[:, :], identb[:, :], IDENT_SCALE)
            nc.vector.memset(ones64[:, :], 1.0)
            nc.gpsimd.memset(Vaug[:, :, D : D + 1], 1.0)
            for t in range(QT_TILES):
                nc.sync.dma_start(PW[t][:, :], projT[t * 128 : (t + 1) * 128, :])

            # ================= Phase 1: QKV projection =================
            with (
                tc.tile_pool(name="ph1", bufs=1) as p1,
                tc.tile_pool(name="ph1p", bufs=8, space="PSUM") as p1p,
            ):
                xts = [
                    p1.tile([128, N], F32R, tag=f"x{ci}", name=f"x{ci}")
                    for ci in range(CIN_T)
                ]
                wts = [
                    p1.tile([128, 3 * HPC * D], F32R, tag=f"w{ci}", name=f"w{ci}")
                    for ci in range(CIN_T)
                ]
                # load order: K weight cols + first x chunk first so the K
                # matmuls can start ~8us in instead of ~13us.
                koff = HPC * D
                for ci in range(CIN_T):
                    nc.scalar.dma_start(
                        wts[ci][:, koff : 2 * koff],
                        qkvT[ci * 128 : (ci + 1) * 128, koff : 2 * koff],
                    )
                    nc.sync.dma_start(
                        xts[ci][:, 0:512], xT[ci * 128 : (ci + 1) * 128, 0:512]
                    )
                for ci in range(CIN_T):
                    nc.scalar.dma_start(
                        wts[ci][:, 0:koff], qkvT[ci * 128 : (ci + 1) * 128, 0:koff]
                    )
                for qc in range(1, 4):
                    for ci in range(CIN_T):
                        nc.sync.dma_start(
                            xts[ci][:, qc * 512 : (qc + 1) * 512],
                            xT[ci * 128 : (ci + 1) * 128, qc * 512 : (qc + 1) * 512],
                        )
                for ci in range(CIN_T):
                    nc.scalar.dma_start(
                        wts[ci][:, 2 * koff : 3 * koff],
                        qkvT[ci * 128 : (ci + 1) * 128, 2 * koff : 3 * koff],
                    )

                # K^T then Q^T production: out[d_tile 128, q 512]
                for which, dst in ((1, KTs), (0, QTs)):
                    off = which * HPC * D  # 0 or 384 within qkvT cols
                    for qc in range(4):
                        for t in range(QT_TILES):
                            ps = p1p.tile([128, 512], F32, tag="p1ps", name="ps")
                            for ci in range(CIN_T):
                                nc.tensor.matmul(
                                    ps[:, :],
                                    wts[ci][:, off + t * 128 : off + (t + 1) * 128],
                                    xts[ci][:, qc * 512 : (qc + 1) * 512],
                                    start=(ci == 0),
                                    stop=(ci == CIN_T - 1),
                                )
                            nc.scalar.copy(
                                dst[t][:, qc * 512 : (qc + 1) * 512], ps[:, :]
                            )

                # V production: out[k_tile 128, 384] -> Vaug f16 (strided per head)
                voff = 2 * HPC * D  # 768
                for kt in range(KT_TILES):
                    ps = p1p.tile([128, HPC * D], F32, tag="p1ps", name="ps")
                    for ci in range(CIN_T):
                        nc.tensor.matmul(
                            ps[:, :],
                            xts[ci][:, kt * 128 : (kt + 1) * 128],
                            wts[ci][:, voff : voff + HPC * D],
                            start=(ci == 0),
                            stop=(ci == CIN_T - 1),
                        )
                    # psum [128, (h 6, d 64)] -> Vaug[:, h*16+kt, 0:64]
                    nc.scalar.copy(
                        Vaug[:, kt :: KT_TILES, 0:D],
                        ps[:, :].rearrange("p (h d) -> p h d", h=HPC),
                    )

            # ================= Phase 2: attention =================
            # Software-pipelined emission: PV groups lag the score/softmax
            # stream by PV_LAG score-groups so the DVE always has reduce work
            # queued while the PE chews through PV/proj matmuls.
            PV_LAG = 3
            with (
                tc.tile_pool(name="mk", bufs=2) as pmk,
                tc.tile_pool(name="work", bufs=2) as pw,
                tc.tile_pool(name="psS", bufs=3, space="PSUM") as psS,
                tc.tile_pool(name="psO", bufs=2, space="PSUM") as psO,
            ):
                mks = {}     # qc -> list of 4 mask tiles
                PT = {}      # (qc, hp) -> [PT0, PT1]

                def load_masks(qc):
                    # one merged load for the q-chunk's 512 mask rows:
                    # dest[p, j, :] = maskb[qc*512 + j*128 + p, :]
                    mkq = pmk.tile([128, 4, N], F8E4, tag="mkq", name="mkq")
                    nc.sync.dma_start(
                        mkq[:, :, :],
                        maskb[qc * 512 : (qc + 1) * 512, :].rearrange(
                            "(j p) n -> p j n", j=4
                        ),
                    )
                    mks[qc] = [mkq[:, j, :] for j in range(4)]

                def score_group(qc, hp, j):
                    if j == 0:
                        PT[(qc, hp)] = [
                            pw.tile(
                                [128, 4, KT_TILES, 128], F16, tag="ptrans",
                                name=f"PT{a}", bufs=4,
                            )
                            for a in range(2)
                        ]
                    PTs = PT[(qc, hp)]
                    qt = qc * 4 + j
                    mstats = pw.tile([128, 4], F32, tag="mstat", name="mstat",
                                     bufs=6)
                    negm = pw.tile([128, 2], F32, tag="negm", name="negm",
                                   bufs=6)
                    fs = pw.tile([128, 2], F32, tag="fs", name="fs", bufs=6)
                    pns = [
                        pw.tile([128, N], F16, tag=f"pn{a}", name=f"pn{a}",
                                bufs=3)
                        for a in range(2)
                    ]
                    for half in range(2):
                        sps = [
                            psS.tile([128, 1024], F32, tag="sp", name=f"sp{a}")
                            for a in range(2)
                        ]
                        # packed K=64 score matmuls: head a in row group a
                        for c in range(2):
                            kc = half * 1024 + c * 512
                            for a in range(2):
                                nc.tensor.matmul(
                                    sps[a][:, c * 512 : (c + 1) * 512],
                                    QTs[hp][
                                        a * D : (a + 1) * D,
                                        qt * 128 : (qt + 1) * 128,
                                    ],
                                    KTs[hp][a * D : (a + 1) * D, kc : kc + 512],
                                    start=True,
                                    stop=False,
                                    tile_position=(a * D, 0),
                                )
                        for c in range(2):
                            kc = half * 1024 + c * 512
                            for a in range(2):
                                nc.tensor.matmul(
                                    sps[a][:, c * 512 : (c + 1) * 512],
                                    ident[:, :],
                                    mks[qc][j][:, kc : kc + 512],
                                    start=False,
                                    stop=True,
                                )
                        for a in range(2):
                            # mstats layout: [a0h0, a0h1, a1h0, a1h1]
                            col = a * 2 + half
                            nc.vector.tensor_reduce(
                                mstats[:, col : col + 1],
                                sps[a][:, :],
                                axis=mybir.AxisListType.X,
                                op=AL.max,
                                negate=True,
                            )
                            if half == 0:
                                # early exp with the half-0 max; fs0
                                # correction lands after negm is known
                                nc.scalar.activation(
                                    pns[a][:, 0:1024],
                                    sps[a][:, :],
                                    mybir.ActivationFunctionType.Exp,
                                    bias=mstats[:, 2 * a : 2 * a + 1],
                                    scale=1.0,
                                )
                        if half == 1:
                            for a in range(2):
                                nc.vector.tensor_reduce(
                                    negm[:, a : a + 1],
                                    mstats[:, 2 * a : 2 * a + 2],
                                    axis=mybir.AxisListType.X,
                                    op=AL.min,
                                )
                                # fs0 = exp(m0 - m) first so the half-0
                                # rescale isn't queued behind the big exp
                                nc.scalar.activation(
                                    fs[:, a : a + 1],
                                    mstats[:, 2 * a : 2 * a + 1],
                                    mybir.ActivationFunctionType.Exp,
                                    bias=negm[:, a : a + 1],
                                    scale=-1.0,
                                )
                                nc.scalar.activation(
                                    pns[a][:, 1024:2048],
                                    sps[a][:, :],
                                    mybir.ActivationFunctionType.Exp,
                                    bias=negm[:, a : a + 1],
                                    scale=1.0,
                                )
                                nc.vector.tensor_scalar(
                                    pns[a][:, 0:1024],
                                    pns[a][:, 0:1024],
                                    fs[:, a : a + 1],
                                    None,
                                    op0=AL.mult,
                                )
                                nc.sync.dma_start_transpose(
                                    PTs[a][:, j, 0:8, :],
                                    pns[a][:, 0:1024],
                                )
                                nc.sync.dma_start_transpose(
                                    PTs[a][:, j, 8:16, :],
                                    pns[a][:, 1024:2048],
                                )

                def pv_group(qc, hp, a):
                    PTs = PT[(qc, hp)]
                    h = 2 * hp + a
                    ht, hpp = hp, a * D
                    # PV: O^T_unnorm [65, 512q]
                    ot = psO.tile([D + 1, 512], F32, tag="ot", name="ot")
                    for kt in range(KT_TILES):
                        nc.tensor.matmul(
                            ot[:, :],
                            Vaug[:, h * KT_TILES + kt, :],
                            PTs[a][:, :, kt, :],
                            start=(kt == 0),
                            stop=(kt == KT_TILES - 1),
                        )
                    rl = pw.tile([1, 512], F32, tag="rl", name="rl", bufs=4)
                    nc.vector.reciprocal(rl[:, :], ot[D : D + 1, :])
                    rb = pw.tile([D, 512], F32, tag="rb", name="rb", bufs=4)
                    nc.gpsimd.partition_broadcast(rb[:, :], rl[:, :])
                    nc.vector.tensor_tensor(
                        Ocat[ht][hpp : hpp + D, qc * 512 : (qc + 1) * 512],
                        ot[0:D, :],
                        rb[:, :],
                        op=AL.mult,
                    )
                    if a == 1:
                        PT.pop((qc, hp))

                def proj_group(qc, j):
                    qt = qc * 4 + j
                    y0 = psO.tile([128, 512], F32, tag="ot", name="y0")
                    y1 = psO.tile([128, 256], F32, tag="ot", name="y1")
                    for ct in range(QT_TILES):
                        lt = Ocat[ct][:, qt * 128 : (qt + 1) * 128]
                        nc.tensor.matmul(
                            y0[:, :], lt, PW[ct][:, 0:512],
                            start=(ct == 0), stop=(ct == QT_TILES - 1),
                        )
                        nc.tensor.matmul(
                            y1[:, :], lt, PW[ct][:, 512:768],
                            start=(ct == 0), stop=(ct == QT_TILES - 1),
                        )
                    ysb = pw.tile([128, C], F32, tag="ysb", name="ysb")
                    nc.scalar.copy(ysb[:, 0:512], y0[:, :])
                    nc.scalar.copy(ysb[:, 512:768], y1[:, :])
                    nc.sync.dma_start(out[qt * 128 : (qt + 1) * 128, :], ysb[:, :])

                # flat schedule: score groups in (qc, hp, j) order; each
                # deferred task fires PV_LAG group-slots after its data is
                # complete.
                groups = [
                    (qc, hp, j)
                    for qc in range(4)
                    for hp in range(QT_TILES)
                    for j in range(4)
                ]
                pending = []  # (due_slot, fn)
                load_masks(0)
                for slot, (qc, hp, j) in enumerate(groups):
                    if j == 0 and hp == 1 and qc + 1 < 4:
                        load_masks(qc + 1)  # prefetch next q-chunk's masks
                    for due, fn in [p for p in pending if p[0] <= slot]:
                        fn()
                        pending.remove((due, fn))
                    score_group(qc, hp, j)
                    if j == 3:
                        # the last head-pair's PV can lag further: its PT ring
                        # slot isn't needed until deep into the next q-chunk
                        lag = PV_LAG
                        for a in range(2):
                            pending.append(
                                (
                                    slot + lag + a,
                                    lambda qc=qc, hp=hp, a=a: pv_group(qc, hp, a),
                                )
                            )
                        if hp == QT_TILES - 1:
                            for jj in range(4):
                                pending.append(
                                    (
                                        slot + lag + 2 + jj,
                                        lambda qc=qc, jj=jj: proj_group(qc, jj),
                                    )
                                )
                for due, fn in sorted(pending, key=lambda p: p[0]):
                    fn()
    nc.compile()
    return nc


def _prepare_in_maps(x, local_attn_mask, qkv_w, proj_w, proj_b):
    x = np.asarray(x, dtype=np.float32)
    mask = np.asarray(local_attn_mask)
    qkv_w = np.asarray(qkv_w, dtype=np.float32)
    proj_w = np.asarray(proj_w, dtype=np.float32)

    maskb = (MASK_FP8_VAL * mask.astype(np.float32)).astype(ml_dtypes.float8_e4m3)
    in_maps = []
    for c in range(NCORES):
        b, hg = c // HG, c % HG
        rq = slice(hg * HPC * D, (hg + 1) * HPC * D)
        rk = slice(C + hg * HPC * D, C + (hg + 1) * HPC * D)
        rv = slice(2 * C + hg * HPC * D, 2 * C + (hg + 1) * HPC * D)
        # softmax scale D folded into the Q weights
        wsel = np.concatenate(
            [qkv_w[rq] * float(D), qkv_w[rk], qkv_w[rv]], axis=0
        )  # [1152, 768]
        in_maps.append(
            {
                "xT": np.ascontiguousarray(x[b].T),
                "qkvT": np.ascontiguousarray(wsel.T),
                "maskb": maskb,
                "projT": np.ascontiguousarray(
                    proj_w[:, hg * HPC * D : (hg + 1) * HPC * D].T
                ).astype(np.float16),
            }
        )
    return in_maps


def kernel(x, local_attn_mask, qkv_w, proj_w, proj_b):
    proj_b = np.asarray(proj_b, dtype=np.float32)
    in_maps = _prepare_in_maps(x, local_attn_mask, qkv_w, proj_w, proj_b)

    if "nc" not in _CACHE:
        _CACHE["nc"] = _build_program()
    res = run_bass_kernel_spmd(
        _CACHE["nc"],
        in_maps,
        core_ids=list(range(NCORES)),
        tmpdir=os.environ.get("KPROF_DIR") or None,
    )
    _CACHE["last_result"] = res
    outs = res.results
    y = np.empty((B, N, C), dtype=np.float32)
    for b in range(B):
        y[b] = outs[2 * b]["out"] + outs[2 * b + 1]["out"] + proj_b[None, :]
    return y

